# revision 30
# baseline (speedup 1.0000x reference)
"""Segment-mean (nn_Center) Trainium2 kernel.

Strategy: shard *classes* across the 8 cores (balanced by row count, <=127
classes per core), and route each input row to the core that owns its class.
Each core computes, fully on device:
    sums[s, :]  = sum of x rows with local class s   (onehot matmul, PSUM acc)
    counts[s]   = number of such rows                (onehot @ ones column)
    out[s, :]   = counts>0 ? sums/counts : class_weight[s, :]
The onehot [128 rows x 128 slots] is built per row-tile on the vector engine
with an iota==target compare; the matmul accumulates across all row tiles
directly in PSUM, so HBM traffic is just one read of the rows.
No cross-core collectives are needed: each core owns its classes end-to-end.
"""

import numpy as np

import concourse.bacc as bacc
import concourse.bass as bass
import concourse.mybir as mybir
import concourse.tile as tile
from concourse.bass_utils import run_bass_kernel_spmd

P = 128
N_CORES = 8
PSUM_BANK_F32 = 512  # one PSUM bank = 512 fp32 = max matmul out width

# Set by each kernel() call: BassKernelResults of the device run (exec_time_ns
# etc. when tracing via BASS_TRACE=1). Used by test.py only.
LAST_RESULTS = None


def _build_nc(T: int, dim: int) -> bass.Bass:
    """Device program for one core: T row-tiles of [128, dim]."""
    nc = bacc.Bacc("TRN2", target_bir_lowering=False)
    x = nc.dram_tensor("x", [T * P, dim], mybir.dt.float32, kind="ExternalInput")
    lcls = nc.dram_tensor("lcls", [P, T], mybir.dt.float32, kind="ExternalInput")
    cw = nc.dram_tensor("cw", [P, dim], mybir.dt.float32, kind="ExternalInput")
    out = nc.dram_tensor("out", [P, dim], mybir.dt.float32, kind="ExternalOutput")

    with tile.TileContext(nc) as tc:
        with (
            tc.tile_pool(name="const", bufs=1) as const_pool,
            tc.tile_pool(name="xp", bufs=12) as x_pool,
            tc.tile_pool(name="xrp", bufs=8) as xr_pool,
            tc.tile_pool(name="lop", bufs=8) as lo_pool,
            tc.tile_pool(name="ohp", bufs=8) as oh_pool,
            tc.tile_pool(name="psum", bufs=1, space="PSUM") as psum_pool,
            tc.tile_pool(name="epi", bufs=1) as epi_pool,
        ):
            # iota[p, m] = m, generated on-device (int iota then cast to f32)
            iota_i = const_pool.tile([P, P], mybir.dt.int32, name="iota_i")
            nc.gpsimd.iota(iota_i[:], pattern=[[1, P]], base=0, channel_multiplier=0)
            iota_t = const_pool.tile([P, P], mybir.dt.float32, name="iota_t")
            nc.vector.tensor_copy(out=iota_t[:], in_=iota_i[:])
            # route lcls through a DVE copy so per-tile onehot compares have
            # engine-local deps only (TensorScalarPtr allows at most 1 wait)
            lcls_in = const_pool.tile([P, T], mybir.dt.float32, name="lcls_in")
            nc.sync.dma_start(out=lcls_in[:], in_=lcls[:, :])
            lcls_t = const_pool.tile([P, T], mybir.dt.float32, name="lcls_t")
            nc.vector.tensor_copy(out=lcls_t[:], in_=lcls_in[:])
            cw_t = const_pool.tile([P, dim], mybir.dt.float32, name="cw_t")
            nc.sync.dma_start(out=cw_t[:], in_=cw[:, :])
            ones_t = const_pool.tile([P, 2], mybir.dt.bfloat16, name="ones_t")
            nc.vector.memset(ones_t[:], 1.0)

            psum_sums = psum_pool.tile(
                [P, dim], mybir.dt.float32, name="psum_sums", space="PSUM"
            )
            psum_cnt = psum_pool.tile(
                [P, 2], mybir.dt.float32, name="psum_cnt", space="PSUM"
            )

            for t in range(T):
                x_t = x_pool.tile([P, dim], mybir.dt.float32, name="x_t")
                nc.sync.dma_start(out=x_t[:], in_=x[t * P : (t + 1) * P, :])
                oh_t = oh_pool.tile([P, P], mybir.dt.bfloat16, name="oh_t")
                nc.vector.tensor_tensor(
                    out=oh_t[:],
                    in0=iota_t[:],
                    in1=lcls_t[:, t : t + 1].to_broadcast([P, P]),
                    op=mybir.AluOpType.is_equal,
                )
                # hi/lo bf16 split: x = hi + lo to ~2^-18 relative; both
                # halves accumulate into the same PSUM, so the matmul path
                # is near-exact while streaming at full bf16 PE rate.
                hi_t = xr_pool.tile([P, dim], mybir.dt.bfloat16, name="hi_t")
                nc.scalar.copy(out=hi_t[:], in_=x_t[:])
                lo_t = lo_pool.tile([P, dim], mybir.dt.bfloat16, name="lo_t")
                # the f32-bf16 subtract is the scarce resource (only DVE and
                # GpSimd can run it); spread it 2:1 across the two engines
                sub_eng = nc.gpsimd if t % 3 == 2 else nc.vector
                sub_eng.tensor_tensor(
                    out=lo_t[:],
                    in0=x_t[:],
                    in1=hi_t[:],
                    op=mybir.AluOpType.subtract,
                )
                first, last = t == 0, t == T - 1
                for j in range(0, dim, PSUM_BANK_F32):
                    nc.tensor.matmul(
                        out=psum_sums[:, j : j + PSUM_BANK_F32],
                        lhsT=oh_t[:],
                        rhs=hi_t[:, j : j + PSUM_BANK_F32],
                        start=first,
                        stop=False,
                    )
                    nc.tensor.matmul(
                        out=psum_sums[:, j : j + PSUM_BANK_F32],
                        lhsT=oh_t[:],
                        rhs=lo_t[:, j : j + PSUM_BANK_F32],
                        start=False,
                        stop=last,
                    )
                nc.tensor.matmul(
                    out=psum_cnt[:, :2],
                    lhsT=oh_t[:],
                    rhs=ones_t[:, :2],
                    start=first,
                    stop=last,
                )

            # counts -> reciprocal of max(counts, 1), presence mask
            cntc = epi_pool.tile([P, 1], mybir.dt.float32, name="cntc")
            nc.vector.tensor_scalar(
                out=cntc[:],
                in0=psum_cnt[:, :1],
                scalar1=1.0,
                scalar2=None,
                op0=mybir.AluOpType.max,
            )
            recip = epi_pool.tile([P, 1], mybir.dt.float32, name="recip")
            nc.vector.reciprocal(out=recip[:], in_=cntc[:])
            mask = epi_pool.tile([P, 1], mybir.dt.uint8, name="mask")
            nc.vector.tensor_scalar(
                out=mask[:],
                in0=psum_cnt[:, :1],
                scalar1=0.5,
                scalar2=None,
                op0=mybir.AluOpType.is_gt,
            )
            means = epi_pool.tile([P, dim], mybir.dt.float32, name="means")
            nc.vector.tensor_scalar(
                out=means[:],
                in0=psum_sums[:, :],
                scalar1=recip[:, :1],
                scalar2=None,
                op0=mybir.AluOpType.mult,
            )
            # overwrite class_weight rows with means where the class is present
            nc.vector.copy_predicated(
                out=cw_t[:],
                mask=mask[:, :1].to_broadcast([P, dim]),
                data=means[:],
            )
            nc.sync.dma_start(out=out[:, :], in_=cw_t[:])
    nc.compile()
    return nc


def kernel(**inputs) -> np.ndarray:
    global LAST_RESULTS
    x = np.ascontiguousarray(np.asarray(inputs["inputs"], dtype=np.float32))
    targets = np.asarray(inputs["targets"]).astype(np.int64).ravel()
    n_classes = int(np.asarray(inputs["classes"]))
    cw = np.ascontiguousarray(np.asarray(inputs["class_weight"], dtype=np.float32))
    n, dim = x.shape

    # --- routing metadata: balanced assignment of classes to cores ---------
    counts = np.bincount(targets, minlength=n_classes)
    order = np.argsort(-counts, kind="stable")
    group_of_class = np.empty(n_classes, dtype=np.int64)
    group_tot = np.zeros(N_CORES, dtype=np.int64)
    group_ncls = np.zeros(N_CORES, dtype=np.int64)
    max_cls = P - 1  # slot 127 reserved as the trash slot for padding rows
    for c in order:
        cand = np.flatnonzero(group_ncls < max_cls)
        g = cand[np.argmin(group_tot[cand])]
        group_of_class[c] = g
        group_tot[g] += counts[c]
        group_ncls[g] += 1

    # refinement: move single classes off the largest group while it helps,
    # to shave padding tiles (n_max -> ceil(n/N_CORES) when possible)
    for _ in range(200):
        g_max = int(np.argmax(group_tot))
        moved = False
        for c in np.flatnonzero(group_of_class == g_max):
            cand = [
                g
                for g in range(N_CORES)
                if g != g_max and group_ncls[g] < max_cls
                and group_tot[g] + counts[c] < group_tot[g_max]
            ]
            if cand:
                g_new = min(cand, key=lambda g: group_tot[g])
                group_of_class[c] = g_new
                group_tot[g_max] -= counts[c]
                group_tot[g_new] += counts[c]
                group_ncls[g_max] -= 1
                group_ncls[g_new] += 1
                moved = True
                break
        if not moved:
            break

    class_slot = np.zeros(n_classes, dtype=np.int64)
    group_classes = []
    for g in range(N_CORES):
        gc = np.flatnonzero(group_of_class == g)
        group_classes.append(gc)
        class_slot[gc] = np.arange(len(gc))

    row_group = group_of_class[targets]
    rows_per = [np.flatnonzero(row_group == g) for g in range(N_CORES)]
    n_max = max(len(r) for r in rows_per)
    T = max(1, (n_max + P - 1) // P)
    pmax = T * P

    in_maps = []
    for g in range(N_CORES):
        r = rows_per[g]
        xg = np.zeros((pmax, dim), dtype=np.float32)
        xg[: len(r)] = x[r]
        lcls = np.full(pmax, P - 1, dtype=np.float32)
        lcls[: len(r)] = class_slot[targets[r]].astype(np.float32)
        lcls2d = np.ascontiguousarray(lcls.reshape(T, P).T)
        cwg = np.zeros((P, dim), dtype=np.float32)
        cwg[: len(group_classes[g])] = cw[group_classes[g]]
        in_maps.append({"x": xg, "lcls": lcls2d, "cw": cwg})

    nc = _build_nc(T, dim)
    res = run_bass_kernel_spmd(nc, in_maps, core_ids=list(range(N_CORES)))
    LAST_RESULTS = res

    out_full = np.empty((n_classes, dim), dtype=np.float32)
    for g in range(N_CORES):
        k = len(group_classes[g])
        out_full[group_classes[g]] = res.results[g]["out"][:k]
    return out_full


# revision 33
# speedup vs baseline: 1.0486x; 1.0486x over previous
"""Segment-mean (nn_Center) Trainium2 kernel.

Strategy: shard *classes* across the 8 cores (balanced by row count, <=127
classes per core), and route each input row to the core that owns its class.
Each core computes, fully on device:
    sums[s, :]  = sum of x rows with local class s   (onehot matmul, PSUM acc)
    counts[s]   = number of such rows                (onehot @ ones column)
    out[s, :]   = counts>0 ? sums/counts : class_weight[s, :]
The onehot [128 rows x 128 slots] is built per row-tile on the vector engine
with an iota==target compare; the matmul accumulates across all row tiles
directly in PSUM, so HBM traffic is just one read of the rows.
No cross-core collectives are needed: each core owns its classes end-to-end.
"""

import numpy as np

import concourse.bacc as bacc
import concourse.bass as bass
import concourse.mybir as mybir
import concourse.tile as tile
from concourse.bass_utils import run_bass_kernel_spmd

P = 128
N_CORES = 8
PSUM_BANK_F32 = 512  # one PSUM bank = 512 fp32 = max matmul out width

# Set by each kernel() call: BassKernelResults of the device run (exec_time_ns
# etc. when tracing via BASS_TRACE=1). Used by test.py only.
LAST_RESULTS = None


def _ensure_axon_ntff_hook():
    """bass_utils' trace path does `from antenv.axon_hooks import ...`, which
    does not exist on some agent images; synthesize it (with the real ctypes
    hook when available, else a None-returning stub that bass_utils handles
    by skipping the trace) so BASS_TRACE=1 can never crash kernel()."""
    import sys
    import types

    try:
        import antenv.axon_hooks  # noqa: F401

        return
    except Exception:
        pass
    hook = None
    try:
        import trn_agent_boot.trn_boot as _tb

        hook = _tb._ntff_profile_via_ctypes("/opt/axon/libaxon_pjrt.so")
    except Exception:
        hook = None
    mod = types.ModuleType("antenv.axon_hooks")
    mod.get_axon_ntff_profile_hook = lambda: hook
    mod.set_axon_ntff_profile_hook = lambda h: None
    try:
        import antenv

        sys.modules["antenv.axon_hooks"] = mod
        antenv.axon_hooks = mod
    except Exception:
        pass


def _build_nc(T: int, dim: int) -> bass.Bass:
    """Device program for one core: T row-tiles of [128, dim]."""
    nc = bacc.Bacc("TRN2", target_bir_lowering=False)
    x = nc.dram_tensor("x", [T * P, dim], mybir.dt.float32, kind="ExternalInput")
    lcls = nc.dram_tensor("lcls", [P, T], mybir.dt.float32, kind="ExternalInput")
    cw = nc.dram_tensor("cw", [P, dim], mybir.dt.float32, kind="ExternalInput")
    out = nc.dram_tensor("out", [P, dim], mybir.dt.float32, kind="ExternalOutput")

    with tile.TileContext(nc) as tc:
        with (
            tc.tile_pool(name="const", bufs=1) as const_pool,
            tc.tile_pool(name="xp", bufs=8) as x_pool,
            tc.tile_pool(name="xrp", bufs=6) as xr_pool,
            tc.tile_pool(name="lop", bufs=6) as lo_pool,
            tc.tile_pool(name="ohp", bufs=6) as oh_pool,
            tc.tile_pool(name="psum", bufs=1, space="PSUM") as psum_pool,
            tc.tile_pool(name="epi", bufs=1) as epi_pool,
        ):
            # iota[p, m] = m, generated on-device (int iota then cast to f32)
            iota_i = const_pool.tile([P, P], mybir.dt.int32, name="iota_i")
            nc.gpsimd.iota(iota_i[:], pattern=[[1, P]], base=0, channel_multiplier=0)
            iota_t = const_pool.tile([P, P], mybir.dt.float32, name="iota_t")
            nc.vector.tensor_copy(out=iota_t[:], in_=iota_i[:])
            # route lcls through a DVE copy so per-tile onehot compares have
            # engine-local deps only (TensorScalarPtr allows at most 1 wait)
            lcls_in = const_pool.tile([P, T], mybir.dt.float32, name="lcls_in")
            nc.sync.dma_start(out=lcls_in[:], in_=lcls[:, :])
            lcls_t = const_pool.tile([P, T], mybir.dt.float32, name="lcls_t")
            nc.vector.tensor_copy(out=lcls_t[:], in_=lcls_in[:])
            cw_t = const_pool.tile([P, dim], mybir.dt.float32, name="cw_t")
            nc.sync.dma_start(out=cw_t[:], in_=cw[:, :])
            ones_t = const_pool.tile([P, 2], mybir.dt.bfloat16, name="ones_t")
            nc.vector.memset(ones_t[:], 1.0)

            psum_sums = psum_pool.tile(
                [P, dim], mybir.dt.float32, name="psum_sums", space="PSUM"
            )
            psum_cnt = psum_pool.tile(
                [P, 2], mybir.dt.float32, name="psum_cnt", space="PSUM"
            )

            for t in range(T):
                x_t = x_pool.tile([P, dim], mybir.dt.float32, name="x_t")
                nc.sync.dma_start(out=x_t[:], in_=x[t * P : (t + 1) * P, :])
                oh_t = oh_pool.tile([P, P], mybir.dt.bfloat16, name="oh_t")
                nc.vector.tensor_scalar(
                    out=oh_t[:],
                    in0=iota_t[:],
                    scalar1=lcls_t[:, t : t + 1],
                    scalar2=None,
                    op0=mybir.AluOpType.is_equal,
                )
                # hi/lo bf16 split: x = hi + lo to ~2^-18 relative; both
                # halves accumulate into the same PSUM, so the matmul path
                # is near-exact while streaming at full bf16 PE rate.
                hi_t = xr_pool.tile([P, dim], mybir.dt.bfloat16, name="hi_t")
                nc.scalar.copy(out=hi_t[:], in_=x_t[:])
                lo_t = lo_pool.tile([P, dim], mybir.dt.bfloat16, name="lo_t")
                # the f32-bf16 subtract is the scarce resource (only DVE and
                # GpSimd can run it); spread it 2:1 across the two engines
                sub_eng = nc.gpsimd if t % 3 == 2 else nc.vector
                sub_eng.tensor_tensor(
                    out=lo_t[:],
                    in0=x_t[:],
                    in1=hi_t[:],
                    op=mybir.AluOpType.subtract,
                )
                first, last = t == 0, t == T - 1
                for j in range(0, dim, PSUM_BANK_F32):
                    nc.tensor.matmul(
                        out=psum_sums[:, j : j + PSUM_BANK_F32],
                        lhsT=oh_t[:],
                        rhs=hi_t[:, j : j + PSUM_BANK_F32],
                        start=first,
                        stop=False,
                    )
                    nc.tensor.matmul(
                        out=psum_sums[:, j : j + PSUM_BANK_F32],
                        lhsT=oh_t[:],
                        rhs=lo_t[:, j : j + PSUM_BANK_F32],
                        start=False,
                        stop=last,
                    )
                nc.tensor.matmul(
                    out=psum_cnt[:, :2],
                    lhsT=oh_t[:],
                    rhs=ones_t[:, :2],
                    start=first,
                    stop=last,
                )

            # counts -> reciprocal of max(counts, 1), presence mask
            cntc = epi_pool.tile([P, 1], mybir.dt.float32, name="cntc")
            nc.vector.tensor_scalar(
                out=cntc[:],
                in0=psum_cnt[:, :1],
                scalar1=1.0,
                scalar2=None,
                op0=mybir.AluOpType.max,
            )
            recip = epi_pool.tile([P, 1], mybir.dt.float32, name="recip")
            nc.vector.reciprocal(out=recip[:], in_=cntc[:])
            mask = epi_pool.tile([P, 1], mybir.dt.uint8, name="mask")
            nc.vector.tensor_scalar(
                out=mask[:],
                in0=psum_cnt[:, :1],
                scalar1=0.5,
                scalar2=None,
                op0=mybir.AluOpType.is_gt,
            )
            means = epi_pool.tile([P, dim], mybir.dt.float32, name="means")
            nc.vector.tensor_scalar(
                out=means[:],
                in0=psum_sums[:, :],
                scalar1=recip[:, :1],
                scalar2=None,
                op0=mybir.AluOpType.mult,
            )
            # overwrite class_weight rows with means where the class is present
            nc.vector.copy_predicated(
                out=cw_t[:],
                mask=mask[:, :1].to_broadcast([P, dim]),
                data=means[:],
            )
            nc.sync.dma_start(out=out[:, :], in_=cw_t[:])
    nc.compile()
    return nc


def kernel(**inputs) -> np.ndarray:
    global LAST_RESULTS
    _ensure_axon_ntff_hook()
    x = np.ascontiguousarray(np.asarray(inputs["inputs"], dtype=np.float32))
    targets = np.asarray(inputs["targets"]).astype(np.int64).ravel()
    n_classes = int(np.asarray(inputs["classes"]))
    cw = np.ascontiguousarray(np.asarray(inputs["class_weight"], dtype=np.float32))
    n, dim = x.shape

    # --- routing metadata: balanced assignment of classes to cores ---------
    counts = np.bincount(targets, minlength=n_classes)
    order = np.argsort(-counts, kind="stable")
    group_of_class = np.empty(n_classes, dtype=np.int64)
    group_tot = np.zeros(N_CORES, dtype=np.int64)
    group_ncls = np.zeros(N_CORES, dtype=np.int64)
    max_cls = P - 1  # slot 127 reserved as the trash slot for padding rows
    for c in order:
        cand = np.flatnonzero(group_ncls < max_cls)
        g = cand[np.argmin(group_tot[cand])]
        group_of_class[c] = g
        group_tot[g] += counts[c]
        group_ncls[g] += 1

    # refinement: move single classes off the largest group while it helps,
    # to shave padding tiles (n_max -> ceil(n/N_CORES) when possible)
    for _ in range(200):
        g_max = int(np.argmax(group_tot))
        moved = False
        for c in np.flatnonzero(group_of_class == g_max):
            cand = [
                g
                for g in range(N_CORES)
                if g != g_max and group_ncls[g] < max_cls
                and group_tot[g] + counts[c] < group_tot[g_max]
            ]
            if cand:
                g_new = min(cand, key=lambda g: group_tot[g])
                group_of_class[c] = g_new
                group_tot[g_max] -= counts[c]
                group_tot[g_new] += counts[c]
                group_ncls[g_max] -= 1
                group_ncls[g_new] += 1
                moved = True
                break
        if not moved:
            break

    class_slot = np.zeros(n_classes, dtype=np.int64)
    group_classes = []
    for g in range(N_CORES):
        gc = np.flatnonzero(group_of_class == g)
        group_classes.append(gc)
        class_slot[gc] = np.arange(len(gc))

    row_group = group_of_class[targets]
    rows_per = [np.flatnonzero(row_group == g) for g in range(N_CORES)]
    n_max = max(len(r) for r in rows_per)
    T = max(1, (n_max + P - 1) // P)
    pmax = T * P

    in_maps = []
    for g in range(N_CORES):
        r = rows_per[g]
        xg = np.zeros((pmax, dim), dtype=np.float32)
        xg[: len(r)] = x[r]
        lcls = np.full(pmax, P - 1, dtype=np.float32)
        lcls[: len(r)] = class_slot[targets[r]].astype(np.float32)
        lcls2d = np.ascontiguousarray(lcls.reshape(T, P).T)
        cwg = np.zeros((P, dim), dtype=np.float32)
        cwg[: len(group_classes[g])] = cw[group_classes[g]]
        in_maps.append({"x": xg, "lcls": lcls2d, "cw": cwg})

    nc = _build_nc(T, dim)
    res = run_bass_kernel_spmd(nc, in_maps, core_ids=list(range(N_CORES)))
    LAST_RESULTS = res

    out_full = np.empty((n_classes, dim), dtype=np.float32)
    for g in range(N_CORES):
        k = len(group_classes[g])
        out_full[group_classes[g]] = res.results[g]["out"][:k]
    return out_full


# revision 38
# speedup vs baseline: 1.1228x; 1.0708x over previous
"""Segment-mean (nn_Center) Trainium2 kernel.

Strategy: shard *classes* across the 8 cores (balanced by row count, <=127
classes per core), and route each input row to the core that owns its class.
Each core computes, fully on device:
    sums[s, :]  = sum of x rows with local class s   (onehot matmul, PSUM acc)
    counts[s]   = number of such rows                (onehot @ ones column)
    out[s, :]   = counts>0 ? sums/counts : class_weight[s, :]
The onehot [128 rows x 128 slots] is built per row-tile on the vector engine
with an iota==target compare; the matmul accumulates across all row tiles
directly in PSUM, so HBM traffic is just one read of the rows.
No cross-core collectives are needed: each core owns its classes end-to-end.
"""

import numpy as np

import concourse.bacc as bacc
import concourse.bass as bass
import concourse.mybir as mybir
import concourse.tile as tile
from concourse.bass_utils import run_bass_kernel_spmd

P = 128
N_CORES = 8
PSUM_BANK_F32 = 512  # one PSUM bank = 512 fp32 = max matmul out width

# Set by each kernel() call: BassKernelResults of the device run (exec_time_ns
# etc. when tracing via BASS_TRACE=1). Used by test.py only.
LAST_RESULTS = None


def _ensure_axon_ntff_hook():
    """bass_utils' trace path does `from antenv.axon_hooks import ...`, which
    does not exist on some agent images; synthesize it (with the real ctypes
    hook when available, else a None-returning stub that bass_utils handles
    by skipping the trace) so BASS_TRACE=1 can never crash kernel()."""
    import sys
    import types

    try:
        import antenv.axon_hooks  # noqa: F401

        return
    except Exception:
        pass
    hook = None
    try:
        import trn_agent_boot.trn_boot as _tb

        hook = _tb._ntff_profile_via_ctypes("/opt/axon/libaxon_pjrt.so")
    except Exception:
        hook = None
    mod = types.ModuleType("antenv.axon_hooks")
    mod.get_axon_ntff_profile_hook = lambda: hook
    mod.set_axon_ntff_profile_hook = lambda h: None
    try:
        import antenv

        sys.modules["antenv.axon_hooks"] = mod
        antenv.axon_hooks = mod
    except Exception:
        pass


def _build_nc(T: int, dim: int) -> bass.Bass:
    """Device program for one core: T row-tiles of [128, dim]."""
    nc = bacc.Bacc("TRN2", target_bir_lowering=False)
    x = nc.dram_tensor("x", [T * P, dim], mybir.dt.float32, kind="ExternalInput")
    lcls = nc.dram_tensor("lcls", [P, T], mybir.dt.float32, kind="ExternalInput")
    cw = nc.dram_tensor("cw", [P, dim], mybir.dt.float32, kind="ExternalInput")
    out = nc.dram_tensor("out", [P, dim], mybir.dt.float32, kind="ExternalOutput")

    with tile.TileContext(nc) as tc:
        with (
            tc.tile_pool(name="const", bufs=1) as const_pool,
            tc.tile_pool(name="xp", bufs=10) as x_pool,
            tc.tile_pool(name="xrp", bufs=8) as xr_pool,
            tc.tile_pool(name="lop", bufs=8) as lo_pool,
            tc.tile_pool(name="ohp", bufs=1) as oh_pool,
            tc.tile_pool(name="psum", bufs=1, space="PSUM") as psum_pool,
            tc.tile_pool(name="epi", bufs=1) as epi_pool,
        ):
            B = 8  # tiles per batched-onehot slab
            n_slabs = (T + B - 1) // B
            # iota8[p, k*128 + m] = m, generated on-device (int iota + cast)
            iota_i = const_pool.tile([P, B * P], mybir.dt.int32, name="iota_i")
            nc.gpsimd.iota(
                iota_i[:].rearrange("p (k m) -> p k m", m=P),
                pattern=[[0, B], [1, P]],
                base=0,
                channel_multiplier=0,
            )
            iota_t = const_pool.tile([P, B * P], mybir.dt.float32, name="iota_t")
            nc.vector.tensor_copy(out=iota_t[:], in_=iota_i[:])
            lcls_in = const_pool.tile([P, T], mybir.dt.float32, name="lcls_in")
            nc.sync.dma_start(out=lcls_in[:], in_=lcls[:, :])
            lcls_t = const_pool.tile([P, T], mybir.dt.float32, name="lcls_t")
            nc.vector.tensor_copy(out=lcls_t[:], in_=lcls_in[:])
            cw_t = const_pool.tile([P, dim], mybir.dt.float32, name="cw_t")
            nc.sync.dma_start(out=cw_t[:], in_=cw[:, :])
            ones_t = const_pool.tile([P, 2], mybir.dt.bfloat16, name="ones_t")
            nc.vector.memset(ones_t[:], 1.0)

            # all onehots depend only on lcls -> hoist them entirely out of
            # the streaming loop (one wide is_equal per 8 tiles, persistent
            # slabs), so the per-tile DVE work is just the lo subtract
            oh_slabs = []
            for s in range(n_slabs):
                r = min(B, T - s * B)
                oh8 = oh_pool.tile([P, B * P], mybir.dt.bfloat16, name=f"oh8_{s}")
                nc.vector.tensor_tensor(
                    out=oh8[:, : r * P].rearrange("p (k m) -> p k m", m=P),
                    in0=iota_t[:, : r * P].rearrange("p (k m) -> p k m", m=P),
                    in1=lcls_t[:, s * B : s * B + r].to_broadcast([P, r, P]),
                    op=mybir.AluOpType.is_equal,
                )
                oh_slabs.append(oh8)

            psum_sums = psum_pool.tile(
                [P, dim], mybir.dt.float32, name="psum_sums", space="PSUM"
            )
            psum_cnt = psum_pool.tile(
                [P, 2], mybir.dt.float32, name="psum_cnt", space="PSUM"
            )

            for t in range(T):
                x_t = x_pool.tile([P, dim], mybir.dt.float32, name="x_t")
                nc.sync.dma_start(out=x_t[:], in_=x[t * P : (t + 1) * P, :])
                oh_t = oh_slabs[t // B][:, (t % B) * P : (t % B + 1) * P]
                # hi/lo bf16 split: x = hi + lo to ~2^-18 relative; both
                # halves accumulate into the same PSUM, so the matmul path
                # is near-exact while streaming at full bf16 PE rate.
                hi_t = xr_pool.tile([P, dim], mybir.dt.bfloat16, name="hi_t")
                nc.scalar.copy(out=hi_t[:], in_=x_t[:])
                lo_t = lo_pool.tile([P, dim], mybir.dt.bfloat16, name="lo_t")
                # the f32-bf16 subtract is the scarce resource (only DVE and
                # GpSimd can run it); spread it 2:1 across the two engines
                sub_eng = nc.gpsimd if t % 3 == 2 else nc.vector
                sub_eng.tensor_tensor(
                    out=lo_t[:],
                    in0=x_t[:],
                    in1=hi_t[:],
                    op=mybir.AluOpType.subtract,
                )
                first, last = t == 0, t == T - 1
                for j in range(0, dim, PSUM_BANK_F32):
                    nc.tensor.matmul(
                        out=psum_sums[:, j : j + PSUM_BANK_F32],
                        lhsT=oh_t,
                        rhs=hi_t[:, j : j + PSUM_BANK_F32],
                        start=first,
                        stop=False,
                    )
                    nc.tensor.matmul(
                        out=psum_sums[:, j : j + PSUM_BANK_F32],
                        lhsT=oh_t,
                        rhs=lo_t[:, j : j + PSUM_BANK_F32],
                        start=False,
                        stop=last,
                    )
                nc.tensor.matmul(
                    out=psum_cnt[:, :2],
                    lhsT=oh_t,
                    rhs=ones_t[:, :2],
                    start=first,
                    stop=last,
                )

            # counts -> reciprocal of max(counts, 1), presence mask
            cntc = epi_pool.tile([P, 1], mybir.dt.float32, name="cntc")
            nc.vector.tensor_scalar(
                out=cntc[:],
                in0=psum_cnt[:, :1],
                scalar1=1.0,
                scalar2=None,
                op0=mybir.AluOpType.max,
            )
            recip = epi_pool.tile([P, 1], mybir.dt.float32, name="recip")
            nc.vector.reciprocal(out=recip[:], in_=cntc[:])
            mask = epi_pool.tile([P, 1], mybir.dt.uint8, name="mask")
            nc.vector.tensor_scalar(
                out=mask[:],
                in0=psum_cnt[:, :1],
                scalar1=0.5,
                scalar2=None,
                op0=mybir.AluOpType.is_gt,
            )
            means = epi_pool.tile([P, dim], mybir.dt.float32, name="means")
            nc.vector.tensor_scalar(
                out=means[:],
                in0=psum_sums[:, :],
                scalar1=recip[:, :1],
                scalar2=None,
                op0=mybir.AluOpType.mult,
            )
            # overwrite class_weight rows with means where the class is present
            nc.vector.copy_predicated(
                out=cw_t[:],
                mask=mask[:, :1].to_broadcast([P, dim]),
                data=means[:],
            )
            nc.sync.dma_start(out=out[:, :], in_=cw_t[:])
    nc.compile()
    return nc


def kernel(**inputs) -> np.ndarray:
    global LAST_RESULTS
    _ensure_axon_ntff_hook()
    x = np.ascontiguousarray(np.asarray(inputs["inputs"], dtype=np.float32))
    targets = np.asarray(inputs["targets"]).astype(np.int64).ravel()
    n_classes = int(np.asarray(inputs["classes"]))
    cw = np.ascontiguousarray(np.asarray(inputs["class_weight"], dtype=np.float32))
    n, dim = x.shape

    # --- routing metadata: balanced assignment of classes to cores ---------
    counts = np.bincount(targets, minlength=n_classes)
    order = np.argsort(-counts, kind="stable")
    group_of_class = np.empty(n_classes, dtype=np.int64)
    group_tot = np.zeros(N_CORES, dtype=np.int64)
    group_ncls = np.zeros(N_CORES, dtype=np.int64)
    max_cls = P - 1  # slot 127 reserved as the trash slot for padding rows
    for c in order:
        cand = np.flatnonzero(group_ncls < max_cls)
        g = cand[np.argmin(group_tot[cand])]
        group_of_class[c] = g
        group_tot[g] += counts[c]
        group_ncls[g] += 1

    # refinement: move single classes off the largest group while it helps,
    # to shave padding tiles (n_max -> ceil(n/N_CORES) when possible)
    for _ in range(200):
        g_max = int(np.argmax(group_tot))
        moved = False
        for c in np.flatnonzero(group_of_class == g_max):
            cand = [
                g
                for g in range(N_CORES)
                if g != g_max and group_ncls[g] < max_cls
                and group_tot[g] + counts[c] < group_tot[g_max]
            ]
            if cand:
                g_new = min(cand, key=lambda g: group_tot[g])
                group_of_class[c] = g_new
                group_tot[g_max] -= counts[c]
                group_tot[g_new] += counts[c]
                group_ncls[g_max] -= 1
                group_ncls[g_new] += 1
                moved = True
                break
        if not moved:
            break

    class_slot = np.zeros(n_classes, dtype=np.int64)
    group_classes = []
    for g in range(N_CORES):
        gc = np.flatnonzero(group_of_class == g)
        group_classes.append(gc)
        class_slot[gc] = np.arange(len(gc))

    row_group = group_of_class[targets]
    rows_per = [np.flatnonzero(row_group == g) for g in range(N_CORES)]
    n_max = max(len(r) for r in rows_per)
    T = max(1, (n_max + P - 1) // P)
    pmax = T * P

    in_maps = []
    for g in range(N_CORES):
        r = rows_per[g]
        xg = np.zeros((pmax, dim), dtype=np.float32)
        xg[: len(r)] = x[r]
        lcls = np.full(pmax, P - 1, dtype=np.float32)
        lcls[: len(r)] = class_slot[targets[r]].astype(np.float32)
        lcls2d = np.ascontiguousarray(lcls.reshape(T, P).T)
        cwg = np.zeros((P, dim), dtype=np.float32)
        cwg[: len(group_classes[g])] = cw[group_classes[g]]
        in_maps.append({"x": xg, "lcls": lcls2d, "cw": cwg})

    nc = _build_nc(T, dim)
    res = run_bass_kernel_spmd(nc, in_maps, core_ids=list(range(N_CORES)))
    LAST_RESULTS = res

    out_full = np.empty((n_classes, dim), dtype=np.float32)
    for g in range(N_CORES):
        k = len(group_classes[g])
        out_full[group_classes[g]] = res.results[g]["out"][:k]
    return out_full


# revision 39
# speedup vs baseline: 1.1583x; 1.0316x over previous
"""Segment-mean (nn_Center) Trainium2 kernel.

Strategy: shard *classes* across the 8 cores (balanced by row count, <=127
classes per core), and route each input row to the core that owns its class.
Each core computes, fully on device:
    sums[s, :]  = sum of x rows with local class s   (onehot matmul, PSUM acc)
    counts[s]   = number of such rows                (onehot @ ones column)
    out[s, :]   = counts>0 ? sums/counts : class_weight[s, :]
The onehot [128 rows x 128 slots] is built per row-tile on the vector engine
with an iota==target compare; the matmul accumulates across all row tiles
directly in PSUM, so HBM traffic is just one read of the rows.
No cross-core collectives are needed: each core owns its classes end-to-end.
"""

import numpy as np

import concourse.bacc as bacc
import concourse.bass as bass
import concourse.mybir as mybir
import concourse.tile as tile
from concourse.bass_utils import run_bass_kernel_spmd

P = 128
N_CORES = 8
PSUM_BANK_F32 = 512  # one PSUM bank = 512 fp32 = max matmul out width

# Set by each kernel() call: BassKernelResults of the device run (exec_time_ns
# etc. when tracing via BASS_TRACE=1). Used by test.py only.
LAST_RESULTS = None


def _ensure_axon_ntff_hook():
    """bass_utils' trace path does `from antenv.axon_hooks import ...`, which
    does not exist on some agent images; synthesize it (with the real ctypes
    hook when available, else a None-returning stub that bass_utils handles
    by skipping the trace) so BASS_TRACE=1 can never crash kernel()."""
    import sys
    import types

    try:
        import antenv.axon_hooks  # noqa: F401

        return
    except Exception:
        pass
    hook = None
    try:
        import trn_agent_boot.trn_boot as _tb

        hook = _tb._ntff_profile_via_ctypes("/opt/axon/libaxon_pjrt.so")
    except Exception:
        hook = None
    mod = types.ModuleType("antenv.axon_hooks")
    mod.get_axon_ntff_profile_hook = lambda: hook
    mod.set_axon_ntff_profile_hook = lambda h: None
    try:
        import antenv

        sys.modules["antenv.axon_hooks"] = mod
        antenv.axon_hooks = mod
    except Exception:
        pass


def _build_nc(T: int, dim: int) -> bass.Bass:
    """Device program for one core: T row-tiles of [128, dim]."""
    nc = bacc.Bacc("TRN2", target_bir_lowering=False)
    x = nc.dram_tensor("x", [T * P, dim], mybir.dt.float32, kind="ExternalInput")
    lcls = nc.dram_tensor("lcls", [P, T], mybir.dt.float32, kind="ExternalInput")
    cw = nc.dram_tensor("cw", [P, dim], mybir.dt.float32, kind="ExternalInput")
    out = nc.dram_tensor("out", [P, dim], mybir.dt.float32, kind="ExternalOutput")

    with tile.TileContext(nc) as tc:
        with (
            tc.tile_pool(name="const", bufs=1) as const_pool,
            tc.tile_pool(name="xp", bufs=10) as x_pool,
            tc.tile_pool(name="xrp", bufs=8) as xr_pool,
            tc.tile_pool(name="lop", bufs=8) as lo_pool,
            tc.tile_pool(name="ohp", bufs=1) as oh_pool,
            tc.tile_pool(name="psum", bufs=1, space="PSUM") as psum_pool,
            tc.tile_pool(name="epi", bufs=1) as epi_pool,
        ):
            B = 8  # tiles per batched-onehot slab
            n_slabs = (T + B - 1) // B
            # iota8[p, k*128 + m] = m, generated on-device (int iota + cast)
            iota_i = const_pool.tile([P, B * P], mybir.dt.int32, name="iota_i")
            nc.gpsimd.iota(
                iota_i[:].rearrange("p (k m) -> p k m", m=P),
                pattern=[[0, B], [1, P]],
                base=0,
                channel_multiplier=0,
            )
            # bf16 iota/lcls (0..127 are exact in bf16): 16-bit inputs let the
            # slab is_equal run in the DVE 2x mode instead of 1x
            iota_t = const_pool.tile([P, B * P], mybir.dt.bfloat16, name="iota_t")
            nc.vector.tensor_copy(out=iota_t[:], in_=iota_i[:])
            lcls_in = const_pool.tile([P, T], mybir.dt.float32, name="lcls_in")
            nc.sync.dma_start(out=lcls_in[:], in_=lcls[:, :])
            lcls_t = const_pool.tile([P, T], mybir.dt.bfloat16, name="lcls_t")
            nc.vector.tensor_copy(out=lcls_t[:], in_=lcls_in[:])
            cw_t = const_pool.tile([P, dim], mybir.dt.float32, name="cw_t")
            nc.sync.dma_start(out=cw_t[:], in_=cw[:, :])
            ones_t = const_pool.tile([P, 2], mybir.dt.bfloat16, name="ones_t")
            nc.vector.memset(ones_t[:], 1.0)

            # all onehots depend only on lcls -> hoist them entirely out of
            # the streaming loop (one wide is_equal per 8 tiles, persistent
            # slabs), so the per-tile DVE work is just the lo subtract
            oh_slabs = []
            for s in range(n_slabs):
                r = min(B, T - s * B)
                oh8 = oh_pool.tile([P, B * P], mybir.dt.bfloat16, name=f"oh8_{s}")
                nc.vector.tensor_tensor(
                    out=oh8[:, : r * P].rearrange("p (k m) -> p k m", m=P),
                    in0=iota_t[:, : r * P].rearrange("p (k m) -> p k m", m=P),
                    in1=lcls_t[:, s * B : s * B + r].to_broadcast([P, r, P]),
                    op=mybir.AluOpType.is_equal,
                )
                oh_slabs.append(oh8)

            psum_sums = psum_pool.tile(
                [P, dim], mybir.dt.float32, name="psum_sums", space="PSUM"
            )
            psum_cnt = psum_pool.tile(
                [P, 2], mybir.dt.float32, name="psum_cnt", space="PSUM"
            )

            for t in range(T):
                x_t = x_pool.tile([P, dim], mybir.dt.float32, name="x_t")
                nc.sync.dma_start(out=x_t[:], in_=x[t * P : (t + 1) * P, :])
                oh_t = oh_slabs[t // B][:, (t % B) * P : (t % B + 1) * P]
                # hi/lo bf16 split: x = hi + lo to ~2^-18 relative; both
                # halves accumulate into the same PSUM, so the matmul path
                # is near-exact while streaming at full bf16 PE rate.
                hi_t = xr_pool.tile([P, dim], mybir.dt.bfloat16, name="hi_t")
                nc.scalar.copy(out=hi_t[:], in_=x_t[:])
                lo_t = lo_pool.tile([P, dim], mybir.dt.bfloat16, name="lo_t")
                # the f32-bf16 subtract is the scarce resource (only DVE and
                # GpSimd can run it); spread it 2:1 across the two engines
                sub_eng = nc.gpsimd if t % 3 == 2 else nc.vector
                sub_eng.tensor_tensor(
                    out=lo_t[:],
                    in0=x_t[:],
                    in1=hi_t[:],
                    op=mybir.AluOpType.subtract,
                )
                first, last = t == 0, t == T - 1
                for j in range(0, dim, PSUM_BANK_F32):
                    nc.tensor.matmul(
                        out=psum_sums[:, j : j + PSUM_BANK_F32],
                        lhsT=oh_t,
                        rhs=hi_t[:, j : j + PSUM_BANK_F32],
                        start=first,
                        stop=False,
                    )
                    nc.tensor.matmul(
                        out=psum_sums[:, j : j + PSUM_BANK_F32],
                        lhsT=oh_t,
                        rhs=lo_t[:, j : j + PSUM_BANK_F32],
                        start=False,
                        stop=last,
                    )
                nc.tensor.matmul(
                    out=psum_cnt[:, :2],
                    lhsT=oh_t,
                    rhs=ones_t[:, :2],
                    start=first,
                    stop=last,
                )

            # counts -> reciprocal of max(counts, 1), presence mask
            cntc = epi_pool.tile([P, 1], mybir.dt.float32, name="cntc")
            nc.vector.tensor_scalar(
                out=cntc[:],
                in0=psum_cnt[:, :1],
                scalar1=1.0,
                scalar2=None,
                op0=mybir.AluOpType.max,
            )
            recip = epi_pool.tile([P, 1], mybir.dt.float32, name="recip")
            nc.vector.reciprocal(out=recip[:], in_=cntc[:])
            mask = epi_pool.tile([P, 1], mybir.dt.uint8, name="mask")
            nc.vector.tensor_scalar(
                out=mask[:],
                in0=psum_cnt[:, :1],
                scalar1=0.5,
                scalar2=None,
                op0=mybir.AluOpType.is_gt,
            )
            means = epi_pool.tile([P, dim], mybir.dt.float32, name="means")
            nc.vector.tensor_scalar(
                out=means[:],
                in0=psum_sums[:, :],
                scalar1=recip[:, :1],
                scalar2=None,
                op0=mybir.AluOpType.mult,
            )
            # overwrite class_weight rows with means where the class is present
            nc.vector.copy_predicated(
                out=cw_t[:],
                mask=mask[:, :1].to_broadcast([P, dim]),
                data=means[:],
            )
            nc.sync.dma_start(out=out[:, :], in_=cw_t[:])
    nc.compile()
    return nc


def kernel(**inputs) -> np.ndarray:
    global LAST_RESULTS
    _ensure_axon_ntff_hook()
    x = np.ascontiguousarray(np.asarray(inputs["inputs"], dtype=np.float32))
    targets = np.asarray(inputs["targets"]).astype(np.int64).ravel()
    n_classes = int(np.asarray(inputs["classes"]))
    cw = np.ascontiguousarray(np.asarray(inputs["class_weight"], dtype=np.float32))
    n, dim = x.shape

    # --- routing metadata: balanced assignment of classes to cores ---------
    counts = np.bincount(targets, minlength=n_classes)
    order = np.argsort(-counts, kind="stable")
    group_of_class = np.empty(n_classes, dtype=np.int64)
    group_tot = np.zeros(N_CORES, dtype=np.int64)
    group_ncls = np.zeros(N_CORES, dtype=np.int64)
    max_cls = P - 1  # slot 127 reserved as the trash slot for padding rows
    for c in order:
        cand = np.flatnonzero(group_ncls < max_cls)
        g = cand[np.argmin(group_tot[cand])]
        group_of_class[c] = g
        group_tot[g] += counts[c]
        group_ncls[g] += 1

    # refinement: move single classes off the largest group while it helps,
    # to shave padding tiles (n_max -> ceil(n/N_CORES) when possible)
    for _ in range(200):
        g_max = int(np.argmax(group_tot))
        moved = False
        for c in np.flatnonzero(group_of_class == g_max):
            cand = [
                g
                for g in range(N_CORES)
                if g != g_max and group_ncls[g] < max_cls
                and group_tot[g] + counts[c] < group_tot[g_max]
            ]
            if cand:
                g_new = min(cand, key=lambda g: group_tot[g])
                group_of_class[c] = g_new
                group_tot[g_max] -= counts[c]
                group_tot[g_new] += counts[c]
                group_ncls[g_max] -= 1
                group_ncls[g_new] += 1
                moved = True
                break
        if not moved:
            break

    class_slot = np.zeros(n_classes, dtype=np.int64)
    group_classes = []
    for g in range(N_CORES):
        gc = np.flatnonzero(group_of_class == g)
        group_classes.append(gc)
        class_slot[gc] = np.arange(len(gc))

    row_group = group_of_class[targets]
    rows_per = [np.flatnonzero(row_group == g) for g in range(N_CORES)]
    n_max = max(len(r) for r in rows_per)
    T = max(1, (n_max + P - 1) // P)
    pmax = T * P

    in_maps = []
    for g in range(N_CORES):
        r = rows_per[g]
        xg = np.zeros((pmax, dim), dtype=np.float32)
        xg[: len(r)] = x[r]
        lcls = np.full(pmax, P - 1, dtype=np.float32)
        lcls[: len(r)] = class_slot[targets[r]].astype(np.float32)
        lcls2d = np.ascontiguousarray(lcls.reshape(T, P).T)
        cwg = np.zeros((P, dim), dtype=np.float32)
        cwg[: len(group_classes[g])] = cw[group_classes[g]]
        in_maps.append({"x": xg, "lcls": lcls2d, "cw": cwg})

    nc = _build_nc(T, dim)
    res = run_bass_kernel_spmd(nc, in_maps, core_ids=list(range(N_CORES)))
    LAST_RESULTS = res

    out_full = np.empty((n_classes, dim), dtype=np.float32)
    for g in range(N_CORES):
        k = len(group_classes[g])
        out_full[group_classes[g]] = res.results[g]["out"][:k]
    return out_full
